# revision 33
# baseline (speedup 1.0000x reference)
"""Trainium2 Bass kernel for nn_CategoricalLinear (MoE-routing batched matvec).

Problem: out[b] = weight[selected_ids[b]] @ x[b]
  x: [2048, 512] f32, selected_ids: [2048] int, weight: [64, 512, 512] f32
  out: [2048, 512] f32

Strategy (category-sharded, NOT the data-parallel hint):
  - Host: stable-sort samples by category; category c's samples become a
    contiguous block, padded to a uniform per-slot capacity PC (48 here).
  - Each of the 8 cores owns 8 categories (the minimal 1/8 slice of the
    weight table) and ALL samples routed to them (~256).
  - float16 data path: weights/x/out all fp16 (fro 3.2e-4 vs f64 — 60x
    inside the 2e-2 gate) halves the dominant weight stream vs f32.
  - Host pre-linearizes every DRAM operand into the SBUF destination
    layout (p-outer, feature i = 4p + s): wt [128, 8, 4, OUT],
    xt [128, 4, NCOL], out [PC, 8, OUT].  Every DMA is then a
    per-partition contiguous copy; the weight stream measured 335 GB/s —
    at the ~332 GB/s effective per-core HBM ceiling.
  - Per category g: PSUM[s, o] += xT[:, k, slot_g]^T @ W_g[:, k, :] over
    the 4 k-chunks (stationary = x columns, moving = weight rows, fp16
    full-rate PE, fp32 PSUM accumulate).
  - Pipelining: weight DMAs double-buffered 2 cats/DMA (wbufs=6 lookahead);
    x tile double-buffered (xbufs=2) so the next body's x load overlaps the
    current tail instead of stalling the weight stream; all 8 output
    blocks packed along the free dim of one SBUF tile -> a single store
    DMA per body.  Steady-state body ~14.6 us ~= DMA bytes (4.2 MB w +
    0.38 MB x + 0.38 MB out) at the HBM ceiling; PE (~6 us) fully hidden.
  - Host: unpad + inverse-permute rows back to the original sample order.

This is better than data-parallel replication: sharding the batch would make
every core read ~the whole 64 MB table (8x the aggregate HBM traffic) and
leaves ~4 samples per (core, category) matmul.
"""

import numpy as np

B, IN, OUT, C = 2048, 512, 512, 64
NCORES = 8
CPC = C // NCORES  # categories per core
KCH = IN // 128  # contraction chunks of 128


def _build_nc(
    PC,
    mm_dtype: str = "float32r",
    loop_iters: int = 0,
    unroll: int = 1,
    wbufs: int = 4,
    cats_per_dma: int = 1,
    interleave: bool = False,
    alt_rings: bool = False,
    split_first: bool = False,
    w_engine: str = "sync",
    merge_xt: bool = False,
    ppbufs: int = 4,
    opbufs: int = 3,
    out_dtype: str = "float32",
    wsplit: int = 1,
    x_engine: str = "scalar",
    o_engine: str = "scalar",
    copy_engine: str = "vector",
    xbufs: int = 1,
    diag: str = "",  # "wonly": weight DMAs only; "noout": skip copy+out
    obatch: int = 1,  # cats per output tile/DMA (obatch*PC <= 128 rows)
    linear: bool = False,  # host pre-linearized DRAM layouts (pure memcpy DMAs)
    stagger: bool = False,  # staggered-reset For_i (pipelines the back-edge)
):
    """Build + compile the SPMD Bass program (same NEFF runs on all 8 cores).

    PC: per-slot sample capacities (even, <= 128) — an int (uniform) or a
        sequence of CPC values. Slot g on every core holds one category
        padded to PC[g] samples.
    loop_iters: if > 0, wrap the body in a device-side For_i loop with
        `unroll` copies of the body per iteration (timing use only).
    """
    import concourse.mybir as mybir
    import concourse.tile as tile
    from concourse import bacc

    f32 = mybir.dt.float32
    mmdt = getattr(mybir.dt, mm_dtype)
    odt = getattr(mybir.dt, out_dtype)
    PCs = [PC] * CPC if isinstance(PC, int) else list(PC)
    assert len(PCs) == CPC
    assert wsplit == 1 or cats_per_dma == 1
    SOFF = [0]
    for p in PCs:
        SOFF.append(SOFF[-1] + p)
    NCOL = SOFF[-1]

    nc = bacc.Bacc(
        "TRN2", target_bir_lowering=False, debug=False, num_devices=NCORES
    )
    if linear:
        # DRAM mirrors the SBUF destination layout (p-outer, feature
        # i = 4p+s): every DMA degenerates to a per-partition contiguous
        # copy (8-32 KB runs) with zero strided descriptors.
        assert all(p == PCs[0] for p in PCs)
        PCU = PCs[0]
        wt = nc.dram_tensor(
            "wt", [128, CPC, KCH, OUT], mmdt, kind="ExternalInput"
        ).ap()
        xt = nc.dram_tensor(
            "xt", [128, KCH, NCOL], mmdt, kind="ExternalInput"
        ).ap()
        out = nc.dram_tensor(
            "out", [PCU, CPC, OUT], odt, kind="ExternalOutput"
        ).ap()
    else:
        wt = nc.dram_tensor(
            "wt", [CPC * IN, OUT], mmdt, kind="ExternalInput"
        ).ap()
        xt = nc.dram_tensor("xt", [IN, NCOL], mmdt, kind="ExternalInput").ap()
        out = nc.dram_tensor("out", [NCOL, OUT], odt, kind="ExternalOutput").ap()

    with tile.TileContext(nc) as tc:
        with (
            tc.tile_pool(name="xp", bufs=xbufs) as xp,
            tc.tile_pool(name="wp", bufs=wbufs) as wp,
            tc.tile_pool(name="pp", bufs=ppbufs, space="PSUM") as pp,
            tc.tile_pool(name="op", bufs=opbufs) as op,
        ):

            def body():
                G = cats_per_dma
                xdma = getattr(nc, x_engine).dma_start
                if diag == "wonly":
                    lhs = None
                elif linear:
                    xtl = xp.tile([128, KCH, NCOL], mmdt, tag="xl")
                    if diag != "nox":  # nox: tile without fill (timing diag)
                        xdma(out=xtl[:], in_=xt[:])
                    lhs = lambda s, g: xtl[:, s, SOFF[g] : SOFF[g] + PCs[g]]
                elif interleave:
                    # p-outer row mapping: partition p holds IN rows
                    # KCH*p + s (s=0..KCH-1). Every DMA is contiguous per
                    # partition (8 KB weight runs, one single xT DMA); the
                    # contraction over s-subsets is a row permutation the
                    # matmul accumulation doesn't care about, as long as x
                    # and W use the same mapping.
                    xt4 = xp.tile([128, KCH, NCOL], mmdt, tag="x4")
                    xdma(
                        out=xt4[:], in_=xt.rearrange("(p s) c -> p s c", p=128)
                    )
                    lhs = lambda s, g: xt4[:, s, SOFF[g] : SOFF[g] + PCs[g]]
                elif merge_xt:
                    # One 3-D DMA for all four k-chunks (same k-outer layout,
                    # one descriptor chain / one fixed cost on the fill path).
                    xt1 = xp.tile([128, KCH, NCOL], mmdt, tag="x1")
                    xdma(
                        out=xt1[:], in_=xt.rearrange("(k p) c -> p k c", p=128)
                    )
                    lhs = lambda s, g: xt1[:, s, SOFF[g] : SOFF[g] + PCs[g]]
                else:
                    xts = []
                    for k in range(KCH):
                        t = xp.tile([128, NCOL], mmdt, tag=f"x{k}")
                        # ACT ring: keep SP HWDGE free for the weight stream
                        xdma(
                            out=t[:], in_=xt[k * 128 : (k + 1) * 128, :]
                        )
                        xts.append(t)
                    lhs = lambda s, g: xts[s][:, SOFF[g] : SOFF[g] + PCs[g]]
                oeng = getattr(nc, o_engine)
                ceng = getattr(nc, copy_engine)
                ndma = 0  # weight-DMA issue counter (for alt_rings)
                for gp in range(0, CPC, G):
                    # Weight block [G cats] as SBUF [128, G, KCH, OUT]. G MB/DMA.
                    wtile = wp.tile([128, G, KCH, OUT], mmdt)
                    if linear:
                        src = wt[:, gp : gp + G, :, :]
                    elif interleave:
                        src = wt[gp * IN : (gp + G) * IN, :].rearrange(
                            "(g p s) o -> p g s o", p=128, s=KCH
                        )
                    else:
                        src = wt[gp * IN : (gp + G) * IN, :].rearrange(
                            "(g k p) o -> p g k o", p=128, k=KCH
                        )

                    def weng():
                        nonlocal ndma
                        ndma += 1
                        return (
                            nc.scalar
                            if (alt_rings and (ndma - 1) % 2)
                            else getattr(nc, w_engine)
                        )

                    first_half = None
                    if wsplit > 1:
                        # Split the per-category weight DMA into wsplit
                        # k-groups (finer DMA/matmul interleaving).
                        kg = KCH // wsplit
                        for h in range(wsplit):
                            if interleave:
                                sub = wt[gp * IN : (gp + 1) * IN, :].rearrange(
                                    "(p s) o -> p s o", p=128
                                )[:, h * kg : (h + 1) * kg, :]
                            else:
                                sub = wt[
                                    gp * IN + h * kg * 128 : gp * IN
                                    + (h + 1) * kg * 128,
                                    :,
                                ].rearrange("(k p) o -> p k o", p=128)
                            weng().dma_start(
                                out=wtile[:, 0, h * kg : (h + 1) * kg, :],
                                in_=sub,
                            )
                    elif split_first and gp == 0 and G == 1:
                        # Halve the fill latency: the first two matmuls only
                        # need k-chunks 0-1, so land them in their own DMA.
                        half = wp.tile([128, 1, KCH // 2, OUT], mmdt, tag="wh")
                        weng().dma_start(
                            out=half[:],
                            in_=wt[0 : IN // 2, :].rearrange(
                                "(g k p) o -> p g k o", p=128, k=KCH // 2
                            ),
                        )
                        weng().dma_start(
                            out=wtile[:, :, KCH // 2 :, :],
                            in_=wt[IN // 2 : IN, :].rearrange(
                                "(g k p) o -> p g k o", p=128, k=KCH // 2
                            ),
                        )
                        first_half = half
                    else:
                        weng().dma_start(out=wtile[:], in_=src)
                    if diag == "wonly":
                        continue
                    for gl in range(G):
                        g = gp + gl
                        ps = pp.tile([PCs[g], OUT], f32, tag="ps")
                        for k in range(KCH):
                            if first_half is not None and k < KCH // 2:
                                rhs = first_half[:, gl, k, :]
                            else:
                                rhs = wtile[:, gl, k, :]
                            nc.tensor.matmul(
                                ps[:],
                                lhsT=lhs(k, g),
                                rhs=rhs,
                                start=(k == 0),
                                stop=(k == KCH - 1),
                            )
                        if diag == "noout":
                            continue
                        # Batch obatch consecutive cats into one SBUF tile
                        # (packed along the free dim — partition offsets on
                        # DVE writes must be 32-aligned, free offsets are
                        # unconstrained) and one 3-dim store DMA.
                        g0 = (g // obatch) * obatch
                        ob = obatch
                        if ob > 1:
                            assert all(p == PCs[0] for p in PCs)
                            if g % ob == 0:
                                body.ot = op.tile(
                                    [PCs[g], ob, OUT], odt, tag="ot"
                                )
                            ceng.tensor_copy(
                                out=body.ot[:, g - g0, :], in_=ps[:]
                            )
                            if g % ob == ob - 1:
                                if linear:
                                    dst = out[:, g0 : g0 + ob, :]
                                else:
                                    dst = out[
                                        SOFF[g0] : SOFF[g0] + ob * PCs[g], :
                                    ].rearrange("(b p) o -> p b o", b=ob)
                                oeng.dma_start(out=dst, in_=body.ot[:])
                        else:
                            ot = op.tile([PCs[g], OUT], odt, tag="ot")
                            ceng.tensor_copy(out=ot[:], in_=ps[:])
                            dst = (
                                out[:, g, :]
                                if linear
                                else out[SOFF[g] : SOFF[g] + PCs[g], :]
                            )
                            oeng.dma_start(out=dst, in_=ot[:])

            if loop_iters > 0:
                if stagger:
                    # 4 reset stages; with unroll=4, one body per stage so
                    # iteration j+1's stage 0 overlaps iteration j's tail
                    # instead of an all-engine barrier at the back-edge.
                    assert unroll == 4
                    with tc.For_i(0, loop_iters, 1, staggered_reset=True):
                        for u in range(unroll):
                            if u:
                                tc.stage_boundary()
                            body()
                else:
                    with tc.For_i(0, loop_iters, 1):
                        for _ in range(unroll):
                            body()
            else:
                for _ in range(unroll):
                    body()
    nc.compile()
    return nc


def _prepare(
    x, selected_ids, weight, mm_dtype="float32r", linear=False, pc_round=16
):
    """Host-side shard prep. Returns (in_maps, meta), or (None, None) when the
    inputs don't fit the compiled layout (handled by the host fallback).

    linear=True emits the pre-linearized layouts matching
    _build_nc(linear=True): wt [128, CPC, KCH, OUT] (p-outer, feature
    i = KCH*p + s), xt [128, KCH, NCOL], out [PC, CPC, OUT]."""
    host_dt = np.float16 if mm_dtype == "float16" else np.float32
    x = np.ascontiguousarray(np.asarray(x, dtype=np.float32))
    ids = np.asarray(selected_ids).astype(np.int64).ravel()
    weight = np.asarray(weight, dtype=np.float32)
    if ids.size != B or ids.min() < 0 or ids.max() >= C:
        return None, None  # out-of-range ids -> host path
    counts = np.bincount(ids, minlength=C)
    mx = int(counts.max())
    if mx > 128 or weight.shape != (C, OUT, IN) or x.shape != (B, IN):
        return None, None  # pathological skew / unexpected shape -> host path
    order = np.argsort(ids, kind="stable")
    x_sorted = x[order]
    offs = np.zeros(C + 1, np.int64)
    offs[1:] = np.cumsum(counts)
    # Identity assignment, uniform capacity rounded to 16. Measured fastest on
    # HW: sorted-assignment layouts with tighter per-slot capacities moved
    # ~0.5 MB/core less but ran 0.9-1.5 us slower (shorter DMA runs / smaller
    # output blocks cost more than the saved bytes). Capacity must be EVEN or
    # the fp32r matmul fast path degrades ~2x (PC=43 measured 58.8 us).
    assign = np.arange(C).reshape(NCORES, CPC).T  # [slot, core] -> category
    r = pc_round
    PCs = [min(128, max(16, (mx + r - 1) // r * r))] * CPC
    SOFF = np.zeros(CPC + 1, np.int64)
    SOFF[1:] = np.cumsum(PCs)
    NCOL = int(SOFF[-1])
    wt_t = np.ascontiguousarray(weight.transpose(0, 2, 1).astype(host_dt))
    in_maps = []
    for core in range(NCORES):
        xt_k = np.zeros((IN, NCOL), host_dt)
        wlist = []
        for g in range(CPC):
            c = int(assign[g, core])
            n = int(counts[c])
            if n:
                xt_k[:, SOFF[g] : SOFF[g] + n] = (
                    x_sorted[offs[c] : offs[c + 1]].T.astype(host_dt)
                )
            wlist.append(wt_t[c])
        if linear:
            # [g][i=4p+s, o] -> [p, g, s, o]
            w_k = np.ascontiguousarray(
                np.stack(wlist, 0)
                .reshape(CPC, 128, KCH, OUT)
                .transpose(1, 0, 2, 3)
            )
            xt_k = np.ascontiguousarray(xt_k.reshape(128, KCH, NCOL))
        else:
            w_k = np.concatenate(wlist, axis=0)  # [CPC*IN, OUT]
        in_maps.append({"wt": w_k, "xt": xt_k})
    meta = dict(
        PCs=PCs, SOFF=SOFF, assign=assign, counts=counts, offs=offs,
        order=order, linear=linear,
    )
    return in_maps, meta


def _gather(results, meta):
    counts, offs, order = meta["counts"], meta["offs"], meta["order"]
    assign, SOFF = meta["assign"], meta["SOFF"]
    out_sorted = np.empty((B, OUT), np.float32)
    for core in range(NCORES):
        o = results[core]["out"]
        for g in range(CPC):
            c = int(assign[g, core])
            n = int(counts[c])
            if n:
                blk = o[:n, g] if meta.get("linear") else o[SOFF[g] : SOFF[g] + n]
                out_sorted[offs[c] : offs[c + 1]] = blk
    out_full = np.empty_like(out_sorted)
    out_full[order] = out_sorted
    return out_full


_LAST = {}  # debug/test introspection: last built nc + shard maps

# Measured-best build config (loop-slope HW timing, 2026-08-08):
#   float16 weights/x (fro 2.5e-4 vs f64 — 80x inside the 2e-2 gate) halve
#   the dominant weight stream vs f32; fp16 output halves the store.
#   linear: host pre-linearizes DRAM to the SBUF destination layout, so
#   every DMA is a per-partition contiguous copy (weight stream measured
#   335 GB/s — at the ~332 GB/s effective HBM ceiling).
#   xbufs=2: double-buffered x tile; body i+1's x DMA overlaps body i's
#   tail instead of stalling the weight stream (-4.5 us, the single
#   biggest win).  cats_per_dma=2: 1 MB weight DMAs.  obatch=8: all 8
#   output blocks packed along the free dim of one SBUF tile -> a single
#   store DMA per body.  Measured 15.9-16.4 us/body (vs 32.2 baseline);
#   body decomposition: weights 12.5 + x 1.1 + out 1.1 + loop-sync ~1.3.
BEST_CFG = dict(
    mm_dtype="float16",
    out_dtype="float16",
    linear=True,
    xbufs=2,
    cats_per_dma=2,
    obatch=8,
    wbufs=6,
    opbufs=2,
)


def kernel(x, selected_ids, weight):
    in_maps, meta = _prepare(
        x,
        selected_ids,
        weight,
        mm_dtype=BEST_CFG["mm_dtype"],
        linear=BEST_CFG.get("linear", False),
    )
    if in_maps is None:
        # Host fallback for inputs outside the compiled layout's assumptions.
        ids = np.asarray(selected_ids).astype(np.int64).ravel()
        w = np.asarray(weight, dtype=np.float32)
        xx = np.asarray(x, dtype=np.float32).reshape(ids.size, -1)
        outf = np.empty((ids.size, w.shape[1]), np.float32)
        for c in np.unique(ids):
            m = ids == c
            outf[m] = xx[m] @ w[c].T
        return outf
    from concourse.bass_utils import run_bass_kernel_spmd

    cfg = dict(BEST_CFG)
    if any(p != meta["PCs"][0] for p in meta["PCs"]):
        cfg["obatch"] = 1  # obatch packing needs uniform slot capacities
    nc = _build_nc(meta["PCs"], **cfg)
    _LAST.update(nc=nc, in_maps=in_maps, meta=meta)
    res = run_bass_kernel_spmd(nc, in_maps, core_ids=list(range(NCORES)))
    return _gather(res.results, meta)



# revision 40
# speedup vs baseline: 1.0149x; 1.0149x over previous
"""Trainium2 Bass kernel for nn_CategoricalLinear (MoE-routing batched matvec).

Problem: out[b] = weight[selected_ids[b]] @ x[b]
  x: [2048, 512] f32, selected_ids: [2048] int, weight: [64, 512, 512] f32
  out: [2048, 512] f32

Strategy (category-sharded, NOT the data-parallel hint):
  - Host: stable-sort samples by category; category c's samples become a
    contiguous block, padded to a uniform per-slot capacity PC (48 here).
  - Each of the 8 cores owns 8 categories (the minimal 1/8 slice of the
    weight table) and ALL samples routed to them (~256).
  - float16 data path: weights/x/out all fp16 (fro 3.2e-4 vs f64 — 60x
    inside the 2e-2 gate) halves the dominant weight stream vs f32.
  - Host pre-linearizes every DRAM operand into the SBUF destination
    layout (p-outer, feature i = 4p + s): wt [128, 8, 4, OUT],
    xt [128, 4, NCOL], out [PC, 8, OUT].  Every DMA is then a
    per-partition contiguous copy; the weight stream measured 335 GB/s —
    at the ~332 GB/s effective per-core HBM ceiling.
  - Per category g: PSUM[s, o] += xT[:, k, slot_g]^T @ W_g[:, k, :] over
    the 4 k-chunks (stationary = x columns, moving = weight rows, fp16
    full-rate PE, fp32 PSUM accumulate).
  - Pipelining: weight DMAs double-buffered 2 cats/DMA (wbufs=6 lookahead);
    x tile double-buffered (xbufs=2) so the next body's x load overlaps the
    current tail instead of stalling the weight stream; all 8 output
    blocks packed along the free dim of one SBUF tile -> a single store
    DMA per body.  Steady-state body ~14.6 us ~= DMA bytes (4.2 MB w +
    0.38 MB x + 0.38 MB out) at the HBM ceiling; PE (~6 us) fully hidden.
  - Host: unpad + inverse-permute rows back to the original sample order.

This is better than data-parallel replication: sharding the batch would make
every core read ~the whole 64 MB table (8x the aggregate HBM traffic) and
leaves ~4 samples per (core, category) matmul.
"""

import numpy as np

B, IN, OUT, C = 2048, 512, 512, 64
NCORES = 8
CPC = C // NCORES  # categories per core
KCH = IN // 128  # contraction chunks of 128


def _build_nc(
    PC,
    mm_dtype: str = "float32r",
    loop_iters: int = 0,
    unroll: int = 1,
    wbufs: int = 4,
    cats_per_dma: int = 1,
    interleave: bool = False,
    alt_rings: bool = False,
    split_first: bool = False,
    w_engine: str = "sync",
    merge_xt: bool = False,
    ppbufs: int = 4,
    opbufs: int = 3,
    out_dtype: str = "float32",
    wsplit: int = 1,
    x_engine: str = "scalar",
    o_engine: str = "scalar",
    copy_engine: str = "vector",
    xbufs: int = 1,
    diag: str = "",  # "wonly": weight DMAs only; "noout": skip copy+out
    obatch: int = 1,  # cats per output tile/DMA (obatch*PC <= 128 rows)
    linear: bool = False,  # host pre-linearized DRAM layouts (pure memcpy DMAs)
    stagger: bool = False,  # staggered-reset For_i (pipelines the back-edge)
    xpipe: bool = False,  # loop mode: body u prefetches body u+1's x tile
    tail_ob2: bool = False,  # loop mode: last body stores in pairs (short tail)
):
    """Build + compile the SPMD Bass program (same NEFF runs on all 8 cores).

    PC: per-slot sample capacities (even, <= 128) — an int (uniform) or a
        sequence of CPC values. Slot g on every core holds one category
        padded to PC[g] samples.
    loop_iters: if > 0, wrap the body in a device-side For_i loop with
        `unroll` copies of the body per iteration (timing use only).
    """
    import concourse.mybir as mybir
    import concourse.tile as tile
    from concourse import bacc

    f32 = mybir.dt.float32
    mmdt = getattr(mybir.dt, mm_dtype)
    odt = getattr(mybir.dt, out_dtype)
    PCs = [PC] * CPC if isinstance(PC, int) else list(PC)
    assert len(PCs) == CPC
    assert wsplit == 1 or cats_per_dma == 1
    SOFF = [0]
    for p in PCs:
        SOFF.append(SOFF[-1] + p)
    NCOL = SOFF[-1]

    nc = bacc.Bacc(
        "TRN2", target_bir_lowering=False, debug=False, num_devices=NCORES
    )
    if linear:
        # DRAM mirrors the SBUF destination layout (p-outer, feature
        # i = 4p+s): every DMA degenerates to a per-partition contiguous
        # copy (8-32 KB runs) with zero strided descriptors.
        assert all(p == PCs[0] for p in PCs)
        PCU = PCs[0]
        wt = nc.dram_tensor(
            "wt", [128, CPC, KCH, OUT], mmdt, kind="ExternalInput"
        ).ap()
        xt = nc.dram_tensor(
            "xt", [128, KCH, NCOL], mmdt, kind="ExternalInput"
        ).ap()
        out = nc.dram_tensor(
            "out", [PCU, CPC, OUT], odt, kind="ExternalOutput"
        ).ap()
    else:
        wt = nc.dram_tensor(
            "wt", [CPC * IN, OUT], mmdt, kind="ExternalInput"
        ).ap()
        xt = nc.dram_tensor("xt", [IN, NCOL], mmdt, kind="ExternalInput").ap()
        out = nc.dram_tensor("out", [NCOL, OUT], odt, kind="ExternalOutput").ap()

    with tile.TileContext(nc) as tc:
        with (
            tc.tile_pool(name="xp", bufs=xbufs) as xp,
            tc.tile_pool(name="wp", bufs=wbufs) as wp,
            tc.tile_pool(name="pp", bufs=ppbufs, space="PSUM") as pp,
            tc.tile_pool(name="op", bufs=opbufs) as op,
        ):

            use_xpipe = xpipe and linear and loop_iters > 0
            if use_xpipe:
                # Two persistent x slots; body u computes from slot u%2 and
                # prefetches slot (u+1)%2 for the next body at its START, so
                # the load overlaps the current body (and the last body's
                # prefetch lands before the For_i barrier — the next
                # iteration opens with x already resident).  unroll must be
                # even so the slot parity wraps consistently.
                assert unroll % 2 == 0 and xbufs >= 2
                XPP = [
                    xp.tile(
                        [128, KCH, NCOL], mmdt, tag=f"xpp{i}", name=f"xpp{i}"
                    )
                    for i in range(2)
                ]
                getattr(nc, x_engine).dma_start(out=XPP[0][:], in_=xt[:])

            def body(u=0):
                G = cats_per_dma
                xdma = getattr(nc, x_engine).dma_start
                if diag == "wonly":
                    lhs = None
                elif use_xpipe:
                    xdma(out=XPP[(u + 1) % 2][:], in_=xt[:])
                    cur = XPP[u % 2]
                    lhs = lambda s, g: cur[:, s, SOFF[g] : SOFF[g] + PCs[g]]
                elif linear:
                    xtl = xp.tile([128, KCH, NCOL], mmdt, tag="xl")
                    if diag != "nox":  # nox: tile without fill (timing diag)
                        xdma(out=xtl[:], in_=xt[:])
                    lhs = lambda s, g: xtl[:, s, SOFF[g] : SOFF[g] + PCs[g]]
                elif interleave:
                    # p-outer row mapping: partition p holds IN rows
                    # KCH*p + s (s=0..KCH-1). Every DMA is contiguous per
                    # partition (8 KB weight runs, one single xT DMA); the
                    # contraction over s-subsets is a row permutation the
                    # matmul accumulation doesn't care about, as long as x
                    # and W use the same mapping.
                    xt4 = xp.tile([128, KCH, NCOL], mmdt, tag="x4")
                    xdma(
                        out=xt4[:], in_=xt.rearrange("(p s) c -> p s c", p=128)
                    )
                    lhs = lambda s, g: xt4[:, s, SOFF[g] : SOFF[g] + PCs[g]]
                elif merge_xt:
                    # One 3-D DMA for all four k-chunks (same k-outer layout,
                    # one descriptor chain / one fixed cost on the fill path).
                    xt1 = xp.tile([128, KCH, NCOL], mmdt, tag="x1")
                    xdma(
                        out=xt1[:], in_=xt.rearrange("(k p) c -> p k c", p=128)
                    )
                    lhs = lambda s, g: xt1[:, s, SOFF[g] : SOFF[g] + PCs[g]]
                else:
                    xts = []
                    for k in range(KCH):
                        t = xp.tile([128, NCOL], mmdt, tag=f"x{k}")
                        # ACT ring: keep SP HWDGE free for the weight stream
                        xdma(
                            out=t[:], in_=xt[k * 128 : (k + 1) * 128, :]
                        )
                        xts.append(t)
                    lhs = lambda s, g: xts[s][:, SOFF[g] : SOFF[g] + PCs[g]]
                oeng = getattr(nc, o_engine)
                ceng = getattr(nc, copy_engine)
                ndma = 0  # weight-DMA issue counter (for alt_rings)
                for gp in range(0, CPC, G):
                    # Weight block [G cats] as SBUF [128, G, KCH, OUT]. G MB/DMA.
                    wtile = wp.tile([128, G, KCH, OUT], mmdt)
                    if linear:
                        src = wt[:, gp : gp + G, :, :]
                    elif interleave:
                        src = wt[gp * IN : (gp + G) * IN, :].rearrange(
                            "(g p s) o -> p g s o", p=128, s=KCH
                        )
                    else:
                        src = wt[gp * IN : (gp + G) * IN, :].rearrange(
                            "(g k p) o -> p g k o", p=128, k=KCH
                        )

                    def weng():
                        nonlocal ndma
                        ndma += 1
                        return (
                            nc.scalar
                            if (alt_rings and (ndma - 1) % 2)
                            else getattr(nc, w_engine)
                        )

                    first_half = None
                    if wsplit > 1:
                        # Split the per-category weight DMA into wsplit
                        # k-groups (finer DMA/matmul interleaving).
                        kg = KCH // wsplit
                        for h in range(wsplit):
                            if interleave:
                                sub = wt[gp * IN : (gp + 1) * IN, :].rearrange(
                                    "(p s) o -> p s o", p=128
                                )[:, h * kg : (h + 1) * kg, :]
                            else:
                                sub = wt[
                                    gp * IN + h * kg * 128 : gp * IN
                                    + (h + 1) * kg * 128,
                                    :,
                                ].rearrange("(k p) o -> p k o", p=128)
                            weng().dma_start(
                                out=wtile[:, 0, h * kg : (h + 1) * kg, :],
                                in_=sub,
                            )
                    elif split_first and gp == 0 and G == 1:
                        # Halve the fill latency: the first two matmuls only
                        # need k-chunks 0-1, so land them in their own DMA.
                        half = wp.tile([128, 1, KCH // 2, OUT], mmdt, tag="wh")
                        weng().dma_start(
                            out=half[:],
                            in_=wt[0 : IN // 2, :].rearrange(
                                "(g k p) o -> p g k o", p=128, k=KCH // 2
                            ),
                        )
                        weng().dma_start(
                            out=wtile[:, :, KCH // 2 :, :],
                            in_=wt[IN // 2 : IN, :].rearrange(
                                "(g k p) o -> p g k o", p=128, k=KCH // 2
                            ),
                        )
                        first_half = half
                    else:
                        weng().dma_start(out=wtile[:], in_=src)
                    if diag == "wonly":
                        continue
                    for gl in range(G):
                        g = gp + gl
                        ps = pp.tile([PCs[g], OUT], f32, tag="ps")
                        for k in range(KCH):
                            if first_half is not None and k < KCH // 2:
                                rhs = first_half[:, gl, k, :]
                            else:
                                rhs = wtile[:, gl, k, :]
                            nc.tensor.matmul(
                                ps[:],
                                lhsT=lhs(k, g),
                                rhs=rhs,
                                start=(k == 0),
                                stop=(k == KCH - 1),
                            )
                        if diag == "noout":
                            continue
                        # Batch obatch consecutive cats into one SBUF tile
                        # (packed along the free dim — partition offsets on
                        # DVE writes must be 32-aligned, free offsets are
                        # unconstrained) and one 3-dim store DMA.
                        ob = obatch
                        if tail_ob2 and u == unroll - 1:
                            ob = min(2, obatch)  # drain last body's stores early
                        g0 = (g // ob) * ob
                        if ob > 1:
                            assert all(p == PCs[0] for p in PCs)
                            if g % ob == 0:
                                body.ot = op.tile(
                                    [PCs[g], ob, OUT], odt, tag="ot"
                                )
                            ceng.tensor_copy(
                                out=body.ot[:, g - g0, :], in_=ps[:]
                            )
                            if g % ob == ob - 1:
                                if linear:
                                    dst = out[:, g0 : g0 + ob, :]
                                else:
                                    dst = out[
                                        SOFF[g0] : SOFF[g0] + ob * PCs[g], :
                                    ].rearrange("(b p) o -> p b o", b=ob)
                                oeng.dma_start(out=dst, in_=body.ot[:])
                        else:
                            ot = op.tile([PCs[g], OUT], odt, tag="ot")
                            ceng.tensor_copy(out=ot[:], in_=ps[:])
                            dst = (
                                out[:, g, :]
                                if linear
                                else out[SOFF[g] : SOFF[g] + PCs[g], :]
                            )
                            oeng.dma_start(out=dst, in_=ot[:])

            if loop_iters > 0:
                if stagger:
                    # 4 reset stages; with unroll=4, one body per stage so
                    # iteration j+1's stage 0 overlaps iteration j's tail
                    # instead of an all-engine barrier at the back-edge.
                    assert unroll == 4
                    with tc.For_i(0, loop_iters, 1, staggered_reset=True):
                        for u in range(unroll):
                            if u:
                                tc.stage_boundary()
                            body(u)
                else:
                    with tc.For_i(0, loop_iters, 1):
                        for u in range(unroll):
                            body(u)
            else:
                for u in range(unroll):
                    body(u)
    nc.compile()
    return nc


def _prepare(
    x, selected_ids, weight, mm_dtype="float32r", linear=False, pc_round=16
):
    """Host-side shard prep. Returns (in_maps, meta), or (None, None) when the
    inputs don't fit the compiled layout (handled by the host fallback).

    linear=True emits the pre-linearized layouts matching
    _build_nc(linear=True): wt [128, CPC, KCH, OUT] (p-outer, feature
    i = KCH*p + s), xt [128, KCH, NCOL], out [PC, CPC, OUT]."""
    host_dt = np.float16 if mm_dtype == "float16" else np.float32
    x = np.ascontiguousarray(np.asarray(x, dtype=np.float32))
    ids = np.asarray(selected_ids).astype(np.int64).ravel()
    weight = np.asarray(weight, dtype=np.float32)
    if ids.size != B or ids.min() < 0 or ids.max() >= C:
        return None, None  # out-of-range ids -> host path
    counts = np.bincount(ids, minlength=C)
    mx = int(counts.max())
    if mx > 128 or weight.shape != (C, OUT, IN) or x.shape != (B, IN):
        return None, None  # pathological skew / unexpected shape -> host path
    order = np.argsort(ids, kind="stable")
    x_sorted = x[order]
    offs = np.zeros(C + 1, np.int64)
    offs[1:] = np.cumsum(counts)
    # Identity assignment, uniform capacity rounded to 16. Measured fastest on
    # HW: sorted-assignment layouts with tighter per-slot capacities moved
    # ~0.5 MB/core less but ran 0.9-1.5 us slower (shorter DMA runs / smaller
    # output blocks cost more than the saved bytes). Capacity must be EVEN or
    # the fp32r matmul fast path degrades ~2x (PC=43 measured 58.8 us).
    assign = np.arange(C).reshape(NCORES, CPC).T  # [slot, core] -> category
    r = pc_round
    PCs = [min(128, max(16, (mx + r - 1) // r * r))] * CPC
    SOFF = np.zeros(CPC + 1, np.int64)
    SOFF[1:] = np.cumsum(PCs)
    NCOL = int(SOFF[-1])
    wt_t = np.ascontiguousarray(weight.transpose(0, 2, 1).astype(host_dt))
    in_maps = []
    for core in range(NCORES):
        xt_k = np.zeros((IN, NCOL), host_dt)
        wlist = []
        for g in range(CPC):
            c = int(assign[g, core])
            n = int(counts[c])
            if n:
                xt_k[:, SOFF[g] : SOFF[g] + n] = (
                    x_sorted[offs[c] : offs[c + 1]].T.astype(host_dt)
                )
            wlist.append(wt_t[c])
        if linear:
            # [g][i=4p+s, o] -> [p, g, s, o]
            w_k = np.ascontiguousarray(
                np.stack(wlist, 0)
                .reshape(CPC, 128, KCH, OUT)
                .transpose(1, 0, 2, 3)
            )
            xt_k = np.ascontiguousarray(xt_k.reshape(128, KCH, NCOL))
        else:
            w_k = np.concatenate(wlist, axis=0)  # [CPC*IN, OUT]
        in_maps.append({"wt": w_k, "xt": xt_k})
    meta = dict(
        PCs=PCs, SOFF=SOFF, assign=assign, counts=counts, offs=offs,
        order=order, linear=linear,
    )
    return in_maps, meta


def _gather(results, meta):
    counts, offs, order = meta["counts"], meta["offs"], meta["order"]
    assign, SOFF = meta["assign"], meta["SOFF"]
    out_sorted = np.empty((B, OUT), np.float32)
    for core in range(NCORES):
        o = results[core]["out"]
        for g in range(CPC):
            c = int(assign[g, core])
            n = int(counts[c])
            if n:
                blk = o[:n, g] if meta.get("linear") else o[SOFF[g] : SOFF[g] + n]
                out_sorted[offs[c] : offs[c + 1]] = blk
    out_full = np.empty_like(out_sorted)
    out_full[order] = out_sorted
    return out_full


_LAST = {}  # debug/test introspection: last built nc + shard maps

# Measured-best build config (loop-slope HW timing, 2026-08-08):
#   float16 weights/x (fro 2.5e-4 vs f64 — 80x inside the 2e-2 gate) halve
#   the dominant weight stream vs f32; fp16 output halves the store.
#   linear: host pre-linearizes DRAM to the SBUF destination layout, so
#   every DMA is a per-partition contiguous copy (weight stream measured
#   335 GB/s — at the ~332 GB/s effective HBM ceiling).
#   xbufs=2: double-buffered x tile; body i+1's x DMA overlaps body i's
#   tail instead of stalling the weight stream (-4.5 us, the single
#   biggest win).  cats_per_dma=2: 1 MB weight DMAs.  obatch=8: all 8
#   output blocks packed along the free dim of one SBUF tile -> a single
#   store DMA per body.  Measured 15.9-16.4 us/body (vs 32.2 baseline);
#   body decomposition: weights 12.5 + x 1.1 + out 1.1 + loop-sync ~1.3.
BEST_CFG = dict(
    mm_dtype="float16",
    out_dtype="float16",
    linear=True,
    xbufs=2,
    cats_per_dma=2,
    obatch=8,
    wbufs=6,
    opbufs=3,
    tail_ob2=True,  # last body stores in pairs: drain overlaps final matmuls
)


def kernel(x, selected_ids, weight):
    in_maps, meta = _prepare(
        x,
        selected_ids,
        weight,
        mm_dtype=BEST_CFG["mm_dtype"],
        linear=BEST_CFG.get("linear", False),
    )
    if in_maps is None:
        # Host fallback for inputs outside the compiled layout's assumptions.
        ids = np.asarray(selected_ids).astype(np.int64).ravel()
        w = np.asarray(weight, dtype=np.float32)
        xx = np.asarray(x, dtype=np.float32).reshape(ids.size, -1)
        outf = np.empty((ids.size, w.shape[1]), np.float32)
        for c in np.unique(ids):
            m = ids == c
            outf[m] = xx[m] @ w[c].T
        return outf
    from concourse.bass_utils import run_bass_kernel_spmd

    cfg = dict(BEST_CFG)
    if any(p != meta["PCs"][0] for p in meta["PCs"]):
        cfg["obatch"] = 1  # obatch packing needs uniform slot capacities
    nc = _build_nc(meta["PCs"], **cfg)
    _LAST.update(nc=nc, in_maps=in_maps, meta=meta)
    res = run_bass_kernel_spmd(nc, in_maps, core_ids=list(range(NCORES)))
    return _gather(res.results, meta)



# revision 42
# speedup vs baseline: 1.0204x; 1.0055x over previous
"""Trainium2 Bass kernel for nn_CategoricalLinear (MoE-routing batched matvec).

Problem: out[b] = weight[selected_ids[b]] @ x[b]
  x: [2048, 512] f32, selected_ids: [2048] int, weight: [64, 512, 512] f32
  out: [2048, 512] f32

Strategy (category-sharded, NOT the data-parallel hint):
  - Host: stable-sort samples by category; category c's samples become a
    contiguous block, padded to a uniform per-slot capacity PC (48 here).
  - Each of the 8 cores owns 8 categories (the minimal 1/8 slice of the
    weight table) and ALL samples routed to them (~256).
  - float16 data path: weights/x/out all fp16 (fro 3.2e-4 vs f64 — 60x
    inside the 2e-2 gate) halves the dominant weight stream vs f32.
  - Host pre-linearizes every DRAM operand into the SBUF destination
    layout (p-outer, feature i = 4p + s): wt [128, 8, 4, OUT],
    xt [128, 4, NCOL], out [PC, 8, OUT].  Every DMA is then a
    per-partition contiguous copy; the weight stream measured 335 GB/s —
    at the ~332 GB/s effective per-core HBM ceiling.
  - Per category g: PSUM[s, o] += xT[:, k, slot_g]^T @ W_g[:, k, :] over
    the 4 k-chunks (stationary = x columns, moving = weight rows, fp16
    full-rate PE, fp32 PSUM accumulate).
  - Pipelining: weight DMAs double-buffered 2 cats/DMA (wbufs=6 lookahead);
    x tile double-buffered (xbufs=2) so the next body's x load overlaps the
    current tail instead of stalling the weight stream; all 8 output
    blocks packed along the free dim of one SBUF tile -> a single store
    DMA per body.  Steady-state body ~14.6 us ~= DMA bytes (4.2 MB w +
    0.38 MB x + 0.38 MB out) at the HBM ceiling; PE (~6 us) fully hidden.
  - Host: unpad + inverse-permute rows back to the original sample order.

This is better than data-parallel replication: sharding the batch would make
every core read ~the whole 64 MB table (8x the aggregate HBM traffic) and
leaves ~4 samples per (core, category) matmul.
"""

import numpy as np

B, IN, OUT, C = 2048, 512, 512, 64
NCORES = 8
CPC = C // NCORES  # categories per core
KCH = IN // 128  # contraction chunks of 128


def _build_nc(
    PC,
    mm_dtype: str = "float32r",
    loop_iters: int = 0,
    unroll: int = 1,
    wbufs: int = 4,
    cats_per_dma: int = 1,
    interleave: bool = False,
    alt_rings: bool = False,
    split_first: bool = False,
    w_engine: str = "sync",
    merge_xt: bool = False,
    ppbufs: int = 4,
    opbufs: int = 3,
    out_dtype: str = "float32",
    wsplit: int = 1,
    x_engine: str = "scalar",
    o_engine: str = "scalar",
    copy_engine: str = "vector",
    xbufs: int = 1,
    diag: str = "",  # "wonly": weight DMAs only; "noout": skip copy+out
    obatch: int = 1,  # cats per output tile/DMA (obatch*PC <= 128 rows)
    linear: bool = False,  # host pre-linearized DRAM layouts (pure memcpy DMAs)
    stagger: bool = False,  # staggered-reset For_i (pipelines the back-edge)
    xpipe: bool = False,  # loop mode: body u prefetches body u+1's x tile
    tail_ob2: bool = False,  # last body stores in pairs (drains under matmuls)
    tail_bodies: int = 1,  # how many trailing bodies get the pair-store
    tail_last_single: bool = False,  # last 2 cats store singly (shorter tail)
):
    """Build + compile the SPMD Bass program (same NEFF runs on all 8 cores).

    PC: per-slot sample capacities (even, <= 128) — an int (uniform) or a
        sequence of CPC values. Slot g on every core holds one category
        padded to PC[g] samples.
    loop_iters: if > 0, wrap the body in a device-side For_i loop with
        `unroll` copies of the body per iteration (timing use only).
    """
    import concourse.mybir as mybir
    import concourse.tile as tile
    from concourse import bacc

    f32 = mybir.dt.float32
    mmdt = getattr(mybir.dt, mm_dtype)
    odt = getattr(mybir.dt, out_dtype)
    PCs = [PC] * CPC if isinstance(PC, int) else list(PC)
    assert len(PCs) == CPC
    assert wsplit == 1 or cats_per_dma == 1
    SOFF = [0]
    for p in PCs:
        SOFF.append(SOFF[-1] + p)
    NCOL = SOFF[-1]

    nc = bacc.Bacc(
        "TRN2", target_bir_lowering=False, debug=False, num_devices=NCORES
    )
    if linear:
        # DRAM mirrors the SBUF destination layout (p-outer, feature
        # i = 4p+s): every DMA degenerates to a per-partition contiguous
        # copy (8-32 KB runs) with zero strided descriptors.
        assert all(p == PCs[0] for p in PCs)
        PCU = PCs[0]
        wt = nc.dram_tensor(
            "wt", [128, CPC, KCH, OUT], mmdt, kind="ExternalInput"
        ).ap()
        xt = nc.dram_tensor(
            "xt", [128, KCH, NCOL], mmdt, kind="ExternalInput"
        ).ap()
        out = nc.dram_tensor(
            "out", [PCU, CPC, OUT], odt, kind="ExternalOutput"
        ).ap()
    else:
        wt = nc.dram_tensor(
            "wt", [CPC * IN, OUT], mmdt, kind="ExternalInput"
        ).ap()
        xt = nc.dram_tensor("xt", [IN, NCOL], mmdt, kind="ExternalInput").ap()
        out = nc.dram_tensor("out", [NCOL, OUT], odt, kind="ExternalOutput").ap()

    with tile.TileContext(nc) as tc:
        with (
            tc.tile_pool(name="xp", bufs=xbufs) as xp,
            tc.tile_pool(name="wp", bufs=wbufs) as wp,
            tc.tile_pool(name="pp", bufs=ppbufs, space="PSUM") as pp,
            tc.tile_pool(name="op", bufs=opbufs) as op,
        ):

            use_xpipe = xpipe and linear and loop_iters > 0
            if use_xpipe:
                # Two persistent x slots; body u computes from slot u%2 and
                # prefetches slot (u+1)%2 for the next body at its START, so
                # the load overlaps the current body (and the last body's
                # prefetch lands before the For_i barrier — the next
                # iteration opens with x already resident).  unroll must be
                # even so the slot parity wraps consistently.
                assert unroll % 2 == 0 and xbufs >= 2
                XPP = [
                    xp.tile(
                        [128, KCH, NCOL], mmdt, tag=f"xpp{i}", name=f"xpp{i}"
                    )
                    for i in range(2)
                ]
                getattr(nc, x_engine).dma_start(out=XPP[0][:], in_=xt[:])

            def body(u=0):
                G = cats_per_dma
                xdma = getattr(nc, x_engine).dma_start
                if diag == "wonly":
                    lhs = None
                elif use_xpipe:
                    xdma(out=XPP[(u + 1) % 2][:], in_=xt[:])
                    cur = XPP[u % 2]
                    lhs = lambda s, g: cur[:, s, SOFF[g] : SOFF[g] + PCs[g]]
                elif linear:
                    xtl = xp.tile([128, KCH, NCOL], mmdt, tag="xl")
                    if diag != "nox":  # nox: tile without fill (timing diag)
                        xdma(out=xtl[:], in_=xt[:])
                    lhs = lambda s, g: xtl[:, s, SOFF[g] : SOFF[g] + PCs[g]]
                elif interleave:
                    # p-outer row mapping: partition p holds IN rows
                    # KCH*p + s (s=0..KCH-1). Every DMA is contiguous per
                    # partition (8 KB weight runs, one single xT DMA); the
                    # contraction over s-subsets is a row permutation the
                    # matmul accumulation doesn't care about, as long as x
                    # and W use the same mapping.
                    xt4 = xp.tile([128, KCH, NCOL], mmdt, tag="x4")
                    xdma(
                        out=xt4[:], in_=xt.rearrange("(p s) c -> p s c", p=128)
                    )
                    lhs = lambda s, g: xt4[:, s, SOFF[g] : SOFF[g] + PCs[g]]
                elif merge_xt:
                    # One 3-D DMA for all four k-chunks (same k-outer layout,
                    # one descriptor chain / one fixed cost on the fill path).
                    xt1 = xp.tile([128, KCH, NCOL], mmdt, tag="x1")
                    xdma(
                        out=xt1[:], in_=xt.rearrange("(k p) c -> p k c", p=128)
                    )
                    lhs = lambda s, g: xt1[:, s, SOFF[g] : SOFF[g] + PCs[g]]
                else:
                    xts = []
                    for k in range(KCH):
                        t = xp.tile([128, NCOL], mmdt, tag=f"x{k}")
                        # ACT ring: keep SP HWDGE free for the weight stream
                        xdma(
                            out=t[:], in_=xt[k * 128 : (k + 1) * 128, :]
                        )
                        xts.append(t)
                    lhs = lambda s, g: xts[s][:, SOFF[g] : SOFF[g] + PCs[g]]
                oeng = getattr(nc, o_engine)
                ceng = getattr(nc, copy_engine)
                ndma = 0  # weight-DMA issue counter (for alt_rings)
                for gp in range(0, CPC, G):
                    # Weight block [G cats] as SBUF [128, G, KCH, OUT]. G MB/DMA.
                    wtile = wp.tile([128, G, KCH, OUT], mmdt)
                    if linear:
                        src = wt[:, gp : gp + G, :, :]
                    elif interleave:
                        src = wt[gp * IN : (gp + G) * IN, :].rearrange(
                            "(g p s) o -> p g s o", p=128, s=KCH
                        )
                    else:
                        src = wt[gp * IN : (gp + G) * IN, :].rearrange(
                            "(g k p) o -> p g k o", p=128, k=KCH
                        )

                    def weng():
                        nonlocal ndma
                        ndma += 1
                        return (
                            nc.scalar
                            if (alt_rings and (ndma - 1) % 2)
                            else getattr(nc, w_engine)
                        )

                    first_half = None
                    if wsplit > 1:
                        # Split the per-category weight DMA into wsplit
                        # k-groups (finer DMA/matmul interleaving).
                        kg = KCH // wsplit
                        for h in range(wsplit):
                            if interleave:
                                sub = wt[gp * IN : (gp + 1) * IN, :].rearrange(
                                    "(p s) o -> p s o", p=128
                                )[:, h * kg : (h + 1) * kg, :]
                            else:
                                sub = wt[
                                    gp * IN + h * kg * 128 : gp * IN
                                    + (h + 1) * kg * 128,
                                    :,
                                ].rearrange("(k p) o -> p k o", p=128)
                            weng().dma_start(
                                out=wtile[:, 0, h * kg : (h + 1) * kg, :],
                                in_=sub,
                            )
                    elif split_first and gp == 0 and G == 1:
                        # Halve the fill latency: the first two matmuls only
                        # need k-chunks 0-1, so land them in their own DMA.
                        half = wp.tile([128, 1, KCH // 2, OUT], mmdt, tag="wh")
                        weng().dma_start(
                            out=half[:],
                            in_=wt[0 : IN // 2, :].rearrange(
                                "(g k p) o -> p g k o", p=128, k=KCH // 2
                            ),
                        )
                        weng().dma_start(
                            out=wtile[:, :, KCH // 2 :, :],
                            in_=wt[IN // 2 : IN, :].rearrange(
                                "(g k p) o -> p g k o", p=128, k=KCH // 2
                            ),
                        )
                        first_half = half
                    else:
                        weng().dma_start(out=wtile[:], in_=src)
                    if diag == "wonly":
                        continue
                    for gl in range(G):
                        g = gp + gl
                        ps = pp.tile([PCs[g], OUT], f32, tag="ps")
                        for k in range(KCH):
                            if first_half is not None and k < KCH // 2:
                                rhs = first_half[:, gl, k, :]
                            else:
                                rhs = wtile[:, gl, k, :]
                            nc.tensor.matmul(
                                ps[:],
                                lhsT=lhs(k, g),
                                rhs=rhs,
                                start=(k == 0),
                                stop=(k == KCH - 1),
                            )
                        if diag == "noout":
                            continue
                        # Batch obatch consecutive cats into one SBUF tile
                        # (packed along the free dim — partition offsets on
                        # DVE writes must be 32-aligned, free offsets are
                        # unconstrained) and one 3-dim store DMA.
                        ob = obatch
                        if tail_ob2 and u >= unroll - tail_bodies:
                            ob = min(2, obatch)  # drain last body's stores early
                            if tail_last_single and u == unroll - 1 and g >= 6:
                                ob = 1
                        g0 = (g // ob) * ob
                        if ob > 1:
                            assert all(p == PCs[0] for p in PCs)
                            if g % ob == 0:
                                body.ot = op.tile(
                                    [PCs[g], ob, OUT], odt, tag="ot"
                                )
                            ceng.tensor_copy(
                                out=body.ot[:, g - g0, :], in_=ps[:]
                            )
                            if g % ob == ob - 1:
                                if linear:
                                    dst = out[:, g0 : g0 + ob, :]
                                else:
                                    dst = out[
                                        SOFF[g0] : SOFF[g0] + ob * PCs[g], :
                                    ].rearrange("(b p) o -> p b o", b=ob)
                                oeng.dma_start(out=dst, in_=body.ot[:])
                        else:
                            ot = op.tile([PCs[g], OUT], odt, tag="ot")
                            ceng.tensor_copy(out=ot[:], in_=ps[:])
                            dst = (
                                out[:, g, :]
                                if linear
                                else out[SOFF[g] : SOFF[g] + PCs[g], :]
                            )
                            oeng.dma_start(out=dst, in_=ot[:])

            if loop_iters > 0:
                if stagger:
                    # 4 reset stages; with unroll=4, one body per stage so
                    # iteration j+1's stage 0 overlaps iteration j's tail
                    # instead of an all-engine barrier at the back-edge.
                    assert unroll == 4
                    with tc.For_i(0, loop_iters, 1, staggered_reset=True):
                        for u in range(unroll):
                            if u:
                                tc.stage_boundary()
                            body(u)
                else:
                    with tc.For_i(0, loop_iters, 1):
                        for u in range(unroll):
                            body(u)
            else:
                for u in range(unroll):
                    body(u)
    nc.compile()
    return nc


def _prepare(
    x, selected_ids, weight, mm_dtype="float32r", linear=False, pc_round=16
):
    """Host-side shard prep. Returns (in_maps, meta), or (None, None) when the
    inputs don't fit the compiled layout (handled by the host fallback).

    linear=True emits the pre-linearized layouts matching
    _build_nc(linear=True): wt [128, CPC, KCH, OUT] (p-outer, feature
    i = KCH*p + s), xt [128, KCH, NCOL], out [PC, CPC, OUT]."""
    host_dt = np.float16 if mm_dtype == "float16" else np.float32
    x = np.ascontiguousarray(np.asarray(x, dtype=np.float32))
    ids = np.asarray(selected_ids).astype(np.int64).ravel()
    weight = np.asarray(weight, dtype=np.float32)
    if ids.size != B or ids.min() < 0 or ids.max() >= C:
        return None, None  # out-of-range ids -> host path
    counts = np.bincount(ids, minlength=C)
    mx = int(counts.max())
    if mx > 128 or weight.shape != (C, OUT, IN) or x.shape != (B, IN):
        return None, None  # pathological skew / unexpected shape -> host path
    order = np.argsort(ids, kind="stable")
    x_sorted = x[order]
    offs = np.zeros(C + 1, np.int64)
    offs[1:] = np.cumsum(counts)
    # Identity assignment, uniform capacity rounded to 16. Measured fastest on
    # HW: sorted-assignment layouts with tighter per-slot capacities moved
    # ~0.5 MB/core less but ran 0.9-1.5 us slower (shorter DMA runs / smaller
    # output blocks cost more than the saved bytes). Capacity must be EVEN or
    # the fp32r matmul fast path degrades ~2x (PC=43 measured 58.8 us).
    assign = np.arange(C).reshape(NCORES, CPC).T  # [slot, core] -> category
    r = pc_round
    PCs = [min(128, max(16, (mx + r - 1) // r * r))] * CPC
    SOFF = np.zeros(CPC + 1, np.int64)
    SOFF[1:] = np.cumsum(PCs)
    NCOL = int(SOFF[-1])
    wt_t = np.ascontiguousarray(weight.transpose(0, 2, 1).astype(host_dt))
    in_maps = []
    for core in range(NCORES):
        xt_k = np.zeros((IN, NCOL), host_dt)
        wlist = []
        for g in range(CPC):
            c = int(assign[g, core])
            n = int(counts[c])
            if n:
                xt_k[:, SOFF[g] : SOFF[g] + n] = (
                    x_sorted[offs[c] : offs[c + 1]].T.astype(host_dt)
                )
            wlist.append(wt_t[c])
        if linear:
            # [g][i=4p+s, o] -> [p, g, s, o]
            w_k = np.ascontiguousarray(
                np.stack(wlist, 0)
                .reshape(CPC, 128, KCH, OUT)
                .transpose(1, 0, 2, 3)
            )
            xt_k = np.ascontiguousarray(xt_k.reshape(128, KCH, NCOL))
        else:
            w_k = np.concatenate(wlist, axis=0)  # [CPC*IN, OUT]
        in_maps.append({"wt": w_k, "xt": xt_k})
    meta = dict(
        PCs=PCs, SOFF=SOFF, assign=assign, counts=counts, offs=offs,
        order=order, linear=linear,
    )
    return in_maps, meta


def _gather(results, meta):
    counts, offs, order = meta["counts"], meta["offs"], meta["order"]
    assign, SOFF = meta["assign"], meta["SOFF"]
    out_sorted = np.empty((B, OUT), np.float32)
    for core in range(NCORES):
        o = results[core]["out"]
        for g in range(CPC):
            c = int(assign[g, core])
            n = int(counts[c])
            if n:
                blk = o[:n, g] if meta.get("linear") else o[SOFF[g] : SOFF[g] + n]
                out_sorted[offs[c] : offs[c + 1]] = blk
    out_full = np.empty_like(out_sorted)
    out_full[order] = out_sorted
    return out_full


_LAST = {}  # debug/test introspection: last built nc + shard maps

# Measured-best build config (loop-slope HW timing, 2026-08-08):
#   float16 weights/x (fro 2.5e-4 vs f64 — 80x inside the 2e-2 gate) halve
#   the dominant weight stream vs f32; fp16 output halves the store.
#   linear: host pre-linearizes DRAM to the SBUF destination layout, so
#   every DMA is a per-partition contiguous copy (weight stream measured
#   335 GB/s — at the ~332 GB/s effective HBM ceiling).
#   xbufs=2: double-buffered x tile; body i+1's x DMA overlaps body i's
#   tail instead of stalling the weight stream (-4.5 us, the single
#   biggest win).  cats_per_dma=2: 1 MB weight DMAs.  obatch=8: all 8
#   output blocks packed along the free dim of one SBUF tile -> a single
#   store DMA per body.  Measured 15.9-16.4 us/body (vs 32.2 baseline);
#   body decomposition: weights 12.5 + x 1.1 + out 1.1 + loop-sync ~1.3.
BEST_CFG = dict(
    mm_dtype="float16",
    out_dtype="float16",
    linear=True,
    xbufs=2,
    cats_per_dma=2,
    obatch=8,
    wbufs=6,
    opbufs=3,
    tail_ob2=True,  # last body stores in pairs: drain overlaps final matmuls
)


def kernel(x, selected_ids, weight):
    in_maps, meta = _prepare(
        x,
        selected_ids,
        weight,
        mm_dtype=BEST_CFG["mm_dtype"],
        linear=BEST_CFG.get("linear", False),
    )
    if in_maps is None:
        # Host fallback for inputs outside the compiled layout's assumptions.
        ids = np.asarray(selected_ids).astype(np.int64).ravel()
        w = np.asarray(weight, dtype=np.float32)
        xx = np.asarray(x, dtype=np.float32).reshape(ids.size, -1)
        outf = np.empty((ids.size, w.shape[1]), np.float32)
        for c in np.unique(ids):
            m = ids == c
            outf[m] = xx[m] @ w[c].T
        return outf
    from concourse.bass_utils import run_bass_kernel_spmd

    cfg = dict(BEST_CFG)
    if any(p != meta["PCs"][0] for p in meta["PCs"]):
        cfg["obatch"] = 1  # obatch packing needs uniform slot capacities
    nc = _build_nc(meta["PCs"], **cfg)
    _LAST.update(nc=nc, in_maps=in_maps, meta=meta)
    res = run_bass_kernel_spmd(nc, in_maps, core_ids=list(range(NCORES)))
    return _gather(res.results, meta)



# revision 46
# speedup vs baseline: 1.0850x; 1.0633x over previous
"""Trainium2 Bass kernel for nn_CategoricalLinear (MoE-routing batched matvec).

Problem: out[b] = weight[selected_ids[b]] @ x[b]
  x: [2048, 512] f32, selected_ids: [2048] int, weight: [64, 512, 512] f32
  out: [2048, 512] f32

Strategy (category-sharded, NOT the data-parallel hint):
  - Host: stable-sort samples by category; category c's samples become a
    contiguous block, padded to a uniform per-slot capacity PC (48 here).
  - Each of the 8 cores owns 8 categories (the minimal 1/8 slice of the
    weight table) and ALL samples routed to them (~256).
  - float16 data path: weights/x/out all fp16 (fro 3.2e-4 vs f64 — 60x
    inside the 2e-2 gate) halves the dominant weight stream vs f32.
  - Host pre-linearizes every DRAM operand into the SBUF destination
    layout (p-outer, feature i = 4p + s): wt [128, 8, 4, OUT],
    xt [128, 4, NCOL], out [PC, 8, OUT].  Every DMA is then a
    per-partition contiguous copy; the weight stream measured 335 GB/s —
    at the ~332 GB/s effective per-core HBM ceiling.
  - Per category g: PSUM[s, o] += xT[:, k, slot_g]^T @ W_g[:, k, :] over
    the 4 k-chunks (stationary = x columns, moving = weight rows, fp16
    full-rate PE, fp32 PSUM accumulate).
  - Pipelining: weight DMAs double-buffered 2 cats/DMA (wbufs=6 lookahead);
    x tile double-buffered (xbufs=2) so the next body's x load overlaps the
    current tail instead of stalling the weight stream; all 8 output
    blocks packed along the free dim of one SBUF tile -> a single store
    DMA per body.  Steady-state body ~14.6 us ~= DMA bytes (4.2 MB w +
    0.38 MB x + 0.38 MB out) at the HBM ceiling; PE (~6 us) fully hidden.
  - Host: unpad + inverse-permute rows back to the original sample order.

This is better than data-parallel replication: sharding the batch would make
every core read ~the whole 64 MB table (8x the aggregate HBM traffic) and
leaves ~4 samples per (core, category) matmul.
"""

import numpy as np

B, IN, OUT, C = 2048, 512, 512, 64
NCORES = 8
CPC = C // NCORES  # categories per core
KCH = IN // 128  # contraction chunks of 128


def _build_nc(
    PC,
    mm_dtype: str = "float32r",
    loop_iters: int = 0,
    unroll: int = 1,
    wbufs: int = 4,
    cats_per_dma: int = 1,
    interleave: bool = False,
    alt_rings: bool = False,
    split_first: bool = False,
    w_engine: str = "sync",
    merge_xt: bool = False,
    ppbufs: int = 4,
    opbufs: int = 3,
    out_dtype: str = "float32",
    wsplit: int = 1,
    x_engine: str = "scalar",
    o_engine: str = "scalar",
    copy_engine: str = "vector",
    xbufs: int = 1,
    diag: str = "",  # "wonly": weight DMAs only; "noout": skip copy+out
    obatch: int = 1,  # cats per output tile/DMA (obatch*PC <= 128 rows)
    linear: bool = False,  # host pre-linearized DRAM layouts (pure memcpy DMAs)
    stagger: bool = False,  # staggered-reset For_i (pipelines the back-edge)
    xpipe: bool = False,  # loop mode: body u prefetches body u+1's x tile
    xwrap: bool = False,  # loop mode: body 0's x loaded in prev body's drain
    tail_ob2: bool = False,  # last body stores in pairs (drains under matmuls)
    tail_bodies: int = 1,  # how many trailing bodies get the pair-store
    tail_last_single: bool = False,  # last 2 cats store singly (shorter tail)
):
    """Build + compile the SPMD Bass program (same NEFF runs on all 8 cores).

    PC: per-slot sample capacities (even, <= 128) — an int (uniform) or a
        sequence of CPC values. Slot g on every core holds one category
        padded to PC[g] samples.
    loop_iters: if > 0, wrap the body in a device-side For_i loop with
        `unroll` copies of the body per iteration (timing use only).
    """
    import concourse.mybir as mybir
    import concourse.tile as tile
    from concourse import bacc

    f32 = mybir.dt.float32
    mmdt = getattr(mybir.dt, mm_dtype)
    odt = getattr(mybir.dt, out_dtype)
    PCs = [PC] * CPC if isinstance(PC, int) else list(PC)
    assert len(PCs) == CPC
    assert wsplit == 1 or cats_per_dma == 1
    SOFF = [0]
    for p in PCs:
        SOFF.append(SOFF[-1] + p)
    NCOL = SOFF[-1]

    nc = bacc.Bacc(
        "TRN2", target_bir_lowering=False, debug=False, num_devices=NCORES
    )
    if linear:
        # DRAM mirrors the SBUF destination layout (p-outer, feature
        # i = 4p+s): every DMA degenerates to a per-partition contiguous
        # copy (8-32 KB runs) with zero strided descriptors.
        assert all(p == PCs[0] for p in PCs)
        PCU = PCs[0]
        wt = nc.dram_tensor(
            "wt", [128, CPC, KCH, OUT], mmdt, kind="ExternalInput"
        ).ap()
        xt = nc.dram_tensor(
            "xt", [128, KCH, NCOL], mmdt, kind="ExternalInput"
        ).ap()
        out = nc.dram_tensor(
            "out", [PCU, CPC, OUT], odt, kind="ExternalOutput"
        ).ap()
    else:
        wt = nc.dram_tensor(
            "wt", [CPC * IN, OUT], mmdt, kind="ExternalInput"
        ).ap()
        xt = nc.dram_tensor("xt", [IN, NCOL], mmdt, kind="ExternalInput").ap()
        out = nc.dram_tensor("out", [NCOL, OUT], odt, kind="ExternalOutput").ap()

    with tile.TileContext(nc) as tc:
        with (
            tc.tile_pool(name="xp", bufs=xbufs) as xp,
            tc.tile_pool(name="wp", bufs=wbufs) as wp,
            tc.tile_pool(name="pp", bufs=ppbufs, space="PSUM") as pp,
            tc.tile_pool(name="op", bufs=opbufs) as op,
        ):

            use_xpipe = xpipe and linear and loop_iters > 0
            use_xw = xwrap and linear and loop_iters > 0
            if use_xw:
                # Body 0 reads a persistent tile; its reload for the next
                # iteration is issued AFTER the last body's stores, so the
                # transfer lands in the drain window (DMA engines otherwise
                # idle) instead of competing with the weight stream after
                # the For_i barrier.
                XW = xp.tile(
                    [128, KCH, NCOL], mmdt, tag="xwrap", name="xwrap"
                )
                getattr(nc, x_engine).dma_start(out=XW[:], in_=xt[:])
            if use_xpipe:
                # Two persistent x slots; body u computes from slot u%2 and
                # prefetches slot (u+1)%2 for the next body at its START, so
                # the load overlaps the current body (and the last body's
                # prefetch lands before the For_i barrier — the next
                # iteration opens with x already resident).  unroll must be
                # even so the slot parity wraps consistently.
                assert unroll % 2 == 0 and xbufs >= 2
                XPP = [
                    xp.tile(
                        [128, KCH, NCOL], mmdt, tag=f"xpp{i}", name=f"xpp{i}"
                    )
                    for i in range(2)
                ]
                getattr(nc, x_engine).dma_start(out=XPP[0][:], in_=xt[:])

            def body(u=0):
                G = cats_per_dma
                xdma = getattr(nc, x_engine).dma_start
                if diag == "wonly":
                    lhs = None
                elif use_xpipe:
                    xdma(out=XPP[(u + 1) % 2][:], in_=xt[:])
                    cur = XPP[u % 2]
                    lhs = lambda s, g: cur[:, s, SOFF[g] : SOFF[g] + PCs[g]]
                elif use_xw and u == 0:
                    lhs = lambda s, g: XW[:, s, SOFF[g] : SOFF[g] + PCs[g]]
                elif linear:
                    xtl = xp.tile([128, KCH, NCOL], mmdt, tag="xl")
                    if diag != "nox":  # nox: tile without fill (timing diag)
                        xdma(out=xtl[:], in_=xt[:])
                    lhs = lambda s, g: xtl[:, s, SOFF[g] : SOFF[g] + PCs[g]]
                elif interleave:
                    # p-outer row mapping: partition p holds IN rows
                    # KCH*p + s (s=0..KCH-1). Every DMA is contiguous per
                    # partition (8 KB weight runs, one single xT DMA); the
                    # contraction over s-subsets is a row permutation the
                    # matmul accumulation doesn't care about, as long as x
                    # and W use the same mapping.
                    xt4 = xp.tile([128, KCH, NCOL], mmdt, tag="x4")
                    xdma(
                        out=xt4[:], in_=xt.rearrange("(p s) c -> p s c", p=128)
                    )
                    lhs = lambda s, g: xt4[:, s, SOFF[g] : SOFF[g] + PCs[g]]
                elif merge_xt:
                    # One 3-D DMA for all four k-chunks (same k-outer layout,
                    # one descriptor chain / one fixed cost on the fill path).
                    xt1 = xp.tile([128, KCH, NCOL], mmdt, tag="x1")
                    xdma(
                        out=xt1[:], in_=xt.rearrange("(k p) c -> p k c", p=128)
                    )
                    lhs = lambda s, g: xt1[:, s, SOFF[g] : SOFF[g] + PCs[g]]
                else:
                    xts = []
                    for k in range(KCH):
                        t = xp.tile([128, NCOL], mmdt, tag=f"x{k}")
                        # ACT ring: keep SP HWDGE free for the weight stream
                        xdma(
                            out=t[:], in_=xt[k * 128 : (k + 1) * 128, :]
                        )
                        xts.append(t)
                    lhs = lambda s, g: xts[s][:, SOFF[g] : SOFF[g] + PCs[g]]
                oeng = getattr(nc, o_engine)
                ceng = getattr(nc, copy_engine)
                ndma = 0  # weight-DMA issue counter (for alt_rings)
                for gp in range(0, CPC, G):
                    # Weight block [G cats] as SBUF [128, G, KCH, OUT]. G MB/DMA.
                    wtile = wp.tile([128, G, KCH, OUT], mmdt)
                    if linear:
                        src = wt[:, gp : gp + G, :, :]
                    elif interleave:
                        src = wt[gp * IN : (gp + G) * IN, :].rearrange(
                            "(g p s) o -> p g s o", p=128, s=KCH
                        )
                    else:
                        src = wt[gp * IN : (gp + G) * IN, :].rearrange(
                            "(g k p) o -> p g k o", p=128, k=KCH
                        )

                    def weng():
                        nonlocal ndma
                        ndma += 1
                        return (
                            nc.scalar
                            if (alt_rings and (ndma - 1) % 2)
                            else getattr(nc, w_engine)
                        )

                    first_half = None
                    if wsplit > 1:
                        # Split the per-category weight DMA into wsplit
                        # k-groups (finer DMA/matmul interleaving).
                        kg = KCH // wsplit
                        for h in range(wsplit):
                            if interleave:
                                sub = wt[gp * IN : (gp + 1) * IN, :].rearrange(
                                    "(p s) o -> p s o", p=128
                                )[:, h * kg : (h + 1) * kg, :]
                            else:
                                sub = wt[
                                    gp * IN + h * kg * 128 : gp * IN
                                    + (h + 1) * kg * 128,
                                    :,
                                ].rearrange("(k p) o -> p k o", p=128)
                            weng().dma_start(
                                out=wtile[:, 0, h * kg : (h + 1) * kg, :],
                                in_=sub,
                            )
                    elif split_first and gp == 0 and G == 1:
                        # Halve the fill latency: the first two matmuls only
                        # need k-chunks 0-1, so land them in their own DMA.
                        half = wp.tile([128, 1, KCH // 2, OUT], mmdt, tag="wh")
                        weng().dma_start(
                            out=half[:],
                            in_=wt[0 : IN // 2, :].rearrange(
                                "(g k p) o -> p g k o", p=128, k=KCH // 2
                            ),
                        )
                        weng().dma_start(
                            out=wtile[:, :, KCH // 2 :, :],
                            in_=wt[IN // 2 : IN, :].rearrange(
                                "(g k p) o -> p g k o", p=128, k=KCH // 2
                            ),
                        )
                        first_half = half
                    else:
                        weng().dma_start(out=wtile[:], in_=src)
                    if diag == "wonly":
                        continue
                    for gl in range(G):
                        g = gp + gl
                        ps = pp.tile([PCs[g], OUT], f32, tag="ps")
                        for k in range(KCH):
                            if first_half is not None and k < KCH // 2:
                                rhs = first_half[:, gl, k, :]
                            else:
                                rhs = wtile[:, gl, k, :]
                            nc.tensor.matmul(
                                ps[:],
                                lhsT=lhs(k, g),
                                rhs=rhs,
                                start=(k == 0),
                                stop=(k == KCH - 1),
                            )
                        if diag == "noout":
                            continue
                        # Batch obatch consecutive cats into one SBUF tile
                        # (packed along the free dim — partition offsets on
                        # DVE writes must be 32-aligned, free offsets are
                        # unconstrained) and one 3-dim store DMA.
                        ob = obatch
                        if tail_ob2 and u >= unroll - tail_bodies:
                            ob = min(2, obatch)  # drain last body's stores early
                            if tail_last_single and u == unroll - 1 and g >= 6:
                                ob = 1
                        g0 = (g // ob) * ob
                        if ob > 1:
                            assert all(p == PCs[0] for p in PCs)
                            if g % ob == 0:
                                body.ot = op.tile(
                                    [PCs[g], ob, OUT], odt, tag="ot"
                                )
                            ceng.tensor_copy(
                                out=body.ot[:, g - g0, :], in_=ps[:]
                            )
                            if g % ob == ob - 1:
                                if linear:
                                    dst = out[:, g0 : g0 + ob, :]
                                else:
                                    dst = out[
                                        SOFF[g0] : SOFF[g0] + ob * PCs[g], :
                                    ].rearrange("(b p) o -> p b o", b=ob)
                                oeng.dma_start(out=dst, in_=body.ot[:])
                        else:
                            ot = op.tile([PCs[g], OUT], odt, tag="ot")
                            ceng.tensor_copy(out=ot[:], in_=ps[:])
                            dst = (
                                out[:, g, :]
                                if linear
                                else out[SOFF[g] : SOFF[g] + PCs[g], :]
                            )
                            oeng.dma_start(out=dst, in_=ot[:])
                if use_xw and u == unroll - 1:
                    # Reload body 0's x for the next iteration in the drain
                    # window (issued after this body's stores).
                    xdma(out=XW[:], in_=xt[:])

            if loop_iters > 0:
                if stagger:
                    # 4 reset stages; with unroll=4, one body per stage so
                    # iteration j+1's stage 0 overlaps iteration j's tail
                    # instead of an all-engine barrier at the back-edge.
                    assert unroll == 4
                    with tc.For_i(0, loop_iters, 1, staggered_reset=True):
                        for u in range(unroll):
                            if u:
                                tc.stage_boundary()
                            body(u)
                else:
                    with tc.For_i(0, loop_iters, 1):
                        for u in range(unroll):
                            body(u)
            else:
                for u in range(unroll):
                    body(u)
    nc.compile()
    return nc


def _prepare(
    x, selected_ids, weight, mm_dtype="float32r", linear=False, pc_round=16
):
    """Host-side shard prep. Returns (in_maps, meta), or (None, None) when the
    inputs don't fit the compiled layout (handled by the host fallback).

    linear=True emits the pre-linearized layouts matching
    _build_nc(linear=True): wt [128, CPC, KCH, OUT] (p-outer, feature
    i = KCH*p + s), xt [128, KCH, NCOL], out [PC, CPC, OUT]."""
    host_dt = np.float16 if mm_dtype == "float16" else np.float32
    x = np.ascontiguousarray(np.asarray(x, dtype=np.float32))
    ids = np.asarray(selected_ids).astype(np.int64).ravel()
    weight = np.asarray(weight, dtype=np.float32)
    if ids.size != B or ids.min() < 0 or ids.max() >= C:
        return None, None  # out-of-range ids -> host path
    counts = np.bincount(ids, minlength=C)
    mx = int(counts.max())
    if mx > 128 or weight.shape != (C, OUT, IN) or x.shape != (B, IN):
        return None, None  # pathological skew / unexpected shape -> host path
    order = np.argsort(ids, kind="stable")
    x_sorted = x[order]
    offs = np.zeros(C + 1, np.int64)
    offs[1:] = np.cumsum(counts)
    # Identity assignment, uniform capacity rounded to 16. Measured fastest on
    # HW: sorted-assignment layouts with tighter per-slot capacities moved
    # ~0.5 MB/core less but ran 0.9-1.5 us slower (shorter DMA runs / smaller
    # output blocks cost more than the saved bytes). Capacity must be EVEN or
    # the fp32r matmul fast path degrades ~2x (PC=43 measured 58.8 us).
    assign = np.arange(C).reshape(NCORES, CPC).T  # [slot, core] -> category
    r = pc_round
    PCs = [min(128, max(16, (mx + r - 1) // r * r))] * CPC
    SOFF = np.zeros(CPC + 1, np.int64)
    SOFF[1:] = np.cumsum(PCs)
    NCOL = int(SOFF[-1])
    wt_t = np.ascontiguousarray(weight.transpose(0, 2, 1).astype(host_dt))
    in_maps = []
    for core in range(NCORES):
        xt_k = np.zeros((IN, NCOL), host_dt)
        wlist = []
        for g in range(CPC):
            c = int(assign[g, core])
            n = int(counts[c])
            if n:
                xt_k[:, SOFF[g] : SOFF[g] + n] = (
                    x_sorted[offs[c] : offs[c + 1]].T.astype(host_dt)
                )
            wlist.append(wt_t[c])
        if linear:
            # [g][i=4p+s, o] -> [p, g, s, o]
            w_k = np.ascontiguousarray(
                np.stack(wlist, 0)
                .reshape(CPC, 128, KCH, OUT)
                .transpose(1, 0, 2, 3)
            )
            xt_k = np.ascontiguousarray(xt_k.reshape(128, KCH, NCOL))
        else:
            w_k = np.concatenate(wlist, axis=0)  # [CPC*IN, OUT]
        in_maps.append({"wt": w_k, "xt": xt_k})
    meta = dict(
        PCs=PCs, SOFF=SOFF, assign=assign, counts=counts, offs=offs,
        order=order, linear=linear,
    )
    return in_maps, meta


def _gather(results, meta):
    counts, offs, order = meta["counts"], meta["offs"], meta["order"]
    assign, SOFF = meta["assign"], meta["SOFF"]
    out_sorted = np.empty((B, OUT), np.float32)
    for core in range(NCORES):
        o = results[core]["out"]
        for g in range(CPC):
            c = int(assign[g, core])
            n = int(counts[c])
            if n:
                blk = o[:n, g] if meta.get("linear") else o[SOFF[g] : SOFF[g] + n]
                out_sorted[offs[c] : offs[c + 1]] = blk
    out_full = np.empty_like(out_sorted)
    out_full[order] = out_sorted
    return out_full


_LAST = {}  # debug/test introspection: last built nc + shard maps

# Measured-best build config (loop-slope HW timing, 2026-08-08):
#   float16 weights/x (fro 2.5e-4 vs f64 — 80x inside the 2e-2 gate) halve
#   the dominant weight stream vs f32; fp16 output halves the store.
#   linear: host pre-linearizes DRAM to the SBUF destination layout, so
#   every DMA is a per-partition contiguous copy (weight stream measured
#   335 GB/s — at the ~332 GB/s effective HBM ceiling).
#   xbufs=2: double-buffered x tile; body i+1's x DMA overlaps body i's
#   tail instead of stalling the weight stream (-4.5 us, the single
#   biggest win).  cats_per_dma=2: 1 MB weight DMAs.  obatch=8: all 8
#   output blocks packed along the free dim of one SBUF tile -> a single
#   store DMA per body.  Measured 15.9-16.4 us/body (vs 32.2 baseline);
#   body decomposition: weights 12.5 + x 1.1 + out 1.1 + loop-sync ~1.3.
BEST_CFG = dict(
    mm_dtype="float16",
    out_dtype="float16",
    linear=True,
    xbufs=2,
    cats_per_dma=2,
    obatch=8,
    wbufs=6,
    opbufs=3,
    tail_ob2=True,  # last body stores in pairs: drain overlaps final matmuls
)


def kernel(x, selected_ids, weight):
    in_maps, meta = _prepare(
        x,
        selected_ids,
        weight,
        mm_dtype=BEST_CFG["mm_dtype"],
        linear=BEST_CFG.get("linear", False),
    )
    if in_maps is None:
        # Host fallback for inputs outside the compiled layout's assumptions.
        ids = np.asarray(selected_ids).astype(np.int64).ravel()
        w = np.asarray(weight, dtype=np.float32)
        xx = np.asarray(x, dtype=np.float32).reshape(ids.size, -1)
        outf = np.empty((ids.size, w.shape[1]), np.float32)
        for c in np.unique(ids):
            m = ids == c
            outf[m] = xx[m] @ w[c].T
        return outf
    from concourse.bass_utils import run_bass_kernel_spmd

    cfg = dict(BEST_CFG)
    if any(p != meta["PCs"][0] for p in meta["PCs"]):
        cfg["obatch"] = 1  # obatch packing needs uniform slot capacities
    nc = _build_nc(meta["PCs"], **cfg)
    _LAST.update(nc=nc, in_maps=in_maps, meta=meta)
    res = run_bass_kernel_spmd(nc, in_maps, core_ids=list(range(NCORES)))
    return _gather(res.results, meta)

